# revision 16
# baseline (speedup 1.0000x reference)
"""Trainium2 Bass kernel for nn_DecoderDynamicTokenSideEmbedding.

Data-parallel across 8 NeuronCores: each core processes 2 batch rows of
[8192] tokens. Full inputs in, full [16, 8192, 768] f32 output back.

Per-core structure (pos-major layout: partition = position-within-block,
free dim = block index, 64 blocks of 128 positions per row):
  1. all constant matrices are host-precomputed and DMA'd in, so gpsimd
     only ever runs the mlp ucode library (no IRAM thrash)
  2. indirect-DMA gathers from the packed [V+NV, 64] f32 table, split
     across the 4 SWDGE queues so all four Q7 core pairs generate
     descriptors concurrently; both rows issued up front
  3. exclusive prefix sums via strict-upper-triangular bf16 matmuls
     (per-block [128x128] + hierarchical block-carry)
  4. family/group running stats selected by one-hot multiply + reduce
     (group-sized ops offloaded to gpsimd, which is idle after gathers)
  5. LayerNorm in f32 (gamma/beta folded into W1 on host)
  6. PE transpose to channel-major bf16, then MLP:
     [38,64] matmul -> exact GELU -> [65,768] matmul (b1/b2/scale folded)
  7. output written to HBM as bf16 (2 blocks per DMA), widened on host

Rows are double-buffered (S/scan/one-hots/feat) and their stages
interleaved so row 1's vector work fills row 0's PE/evacuation phases.
"""

import os

import numpy as np
import ml_dtypes

import concourse.bass as bass
import concourse.bacc as bacc
import concourse.tile as tile
import concourse.mybir as mybir
from concourse.library_config import mlp as _mlp_lib

F32 = mybir.dt.float32
BF16 = mybir.dt.bfloat16
I32 = mybir.dt.int32
I16 = mybir.dt.int16
OP = mybir.AluOpType
ACTF = mybir.ActivationFunctionType

P = 128
NF, NG = 16, 32
C = 2 + 2 * NF + 2 * NG  # 98 scan channels
FD = 37                  # true feature count
H, D = 64, 768
V, NV = 32000, 512
EPS = 1e-5
NQ = 4                   # SWDGE queues used for gathers

# scan channel layout
SC_VF, SC_LV = 0, 1
SC_FV, SC_FL = 2, 2 + NF            # fam_valid [2:18), fam_logv [18:34)
SC_GV, SC_GL = 2 + 2 * NF, 2 + 2 * NF + NG  # [34:66), [66:98)

# packed host-constant layouts (free-dim offsets)
#   cst_f32 [P, 128+128+NB+1]: shift1, e2m, posn, eps
#   cst_b16 [P, 128+128+NB+NF+NG+1+128]: tri128, iden, tri64, iotf, iotg,
#                                        ones_col, ones_nb

# device feature layout (feat tile channel index)
# 0 has_int, 1 tok_log, 2 tok_signed, 3 zero, 4 one, 5 pow2,
# 6:22 fam_oh, 22 outer, 23 inner, 24 has_outer, 25 has_inner,
# 26 pos_norm, 27 prev_count_n, 28 prev_logsum_n,
# 29 psf_c_n, 30 psf_l_n, 31 psg_c_n, 32 psg_l_n,
# 33 psf_ratio, 34 psg_ratio, 35 prev_tok_log, 36 prev_tok_signed
REF_PERM = ([0, 2, 1, 3, 4, 5] + list(range(6, 22)) + [22, 23, 24, 25]
            + [26, 27, 28, 29, 30, 32, 33, 31, 34, 35, 36])


def emit(tc, ins, outs, R, L):
    """Emit the per-core program. ins/outs: dicts of DRAM APs."""
    nc = tc.nc
    NB = L // P
    denom = float(max(L - 1, 1))
    gidx_d, gtable = ins["gidx"], ins["gtable"]
    w1e_d, w2e_d = ins["w1e"], ins["w2e"]
    cf_d, cb_d = ins["cst_f32"], ins["cst_b16"]
    out_d = outs["out"]
    NI = 2 * L
    NIQ = NI // NQ       # indices per gather-queue chunk
    BC = NIQ // P        # gv columns per chunk
    NBUF = min(2, R)

    with (
        tc.tile_pool(name="const", bufs=1) as cp,
        tc.tile_pool(name="row", bufs=1) as rp,
        tc.tile_pool(name="io", bufs=2) as iop,
        tc.tile_pool(name="pg", bufs=1) as pgp,
        tc.tile_pool(name="osb", bufs=3) as op_,
        tc.tile_pool(name="pmisc", bufs=3, space="PSUM") as pmisc,
        tc.tile_pool(name="ptot", bufs=1, space="PSUM") as ptot,
        tc.tile_pool(name="pmm2", bufs=4, space="PSUM") as pmm2,
    ):
        nc.gpsimd.load_library(_mlp_lib)

        # ---- input DMAs ----
        gix = cp.tile([P, R, NIQ // 16], I16)
        nc.sync.dma_start(out=gix[:, :, :], in_=gidx_d[:, :, :])
        w1e = cp.tile([FD + 1, H], BF16)
        nc.sync.dma_start(out=w1e[:, :], in_=w1e_d[:, :])
        w2e = cp.tile([H + 1, D], BF16)
        nc.sync.dma_start(out=w2e[:, :], in_=w2e_d[:, :])
        FTOT = P + P + NB + 1
        cst_f = cp.tile([P, FTOT], F32)
        nc.sync.dma_start(out=cst_f[:, :], in_=cf_d[:, :])
        BTOT = P + P + NB + NF + NG + 1 + P
        cst_b = cp.tile([P, BTOT], BF16)
        nc.sync.dma_start(out=cst_b[:, :], in_=cb_d[:, :])

        shift1 = cst_f[:, 0:P]
        e2m = cst_f[:, P:2 * P]
        posn = cst_f[:, 2 * P:2 * P + NB]
        eps_t = cst_f[:, 2 * P + NB:2 * P + NB + 1]
        o = 0
        tri128 = cst_b[:, o:o + P]; o += P
        iden = cst_b[:, o:o + P]; o += P
        tri64 = cst_b[0:NB, o:o + NB]; o += NB
        iotf = cst_b[:, o:o + NF]; o += NF
        iotg = cst_b[:, o:o + NG]; o += NG
        ones_col = cst_b[:, o:o + 1]; o += 1
        ones_nb = cst_b[0:NB, o:o + P]; o += P

        # ---- all gathers up front: 4 SWDGE queues x R rows ----
        gv = rp.tile([P, R, 2 * NB, 64], F32)
        for r in range(R):
            for q in range(NQ):
                nc.gpsimd.dma_gather(
                    gv[:, r, q * BC:(q + 1) * BC, :], gtable[:, :],
                    gix[:, r, :], NIQ, NIQ, 64,
                    single_packet=False, queue_num=q)

        # ---- persistent row tiles (double-buffered where rows overlap) ----
        S = [rp.tile([P, NB, C], BF16, name=f"S{i}", tag=f"S{i}") for i in range(NBUF)]
        goh = [rp.tile([P, NB, NG], BF16, name=f"goh{i}", tag=f"goh{i}")
               for i in range(NBUF)]
        famoh = [rp.tile([P, NB, NF], BF16, name=f"famoh{i}", tag=f"famoh{i}")
                 for i in range(NBUF)]
        scan_sb = [rp.tile([P, NB, C], BF16, name=f"scan{i}", tag=f"scan{i}")
                   for i in range(NBUF)]
        feat = [rp.tile([P, NB, FD], F32, name=f"feat{i}", tag=f"feat{i}")
                for i in range(NBUF)]
        sqf = rp.tile([P, NB, FD], F32)
        xhb = [rp.tile([P, NB, FD + 1], BF16, name=f"xhb{i}", tag=f"xhb{i}")
               for i in range(NBUF)]
        for i in range(NBUF):
            nc.gpsimd.memset(xhb[i][:, :, FD:FD + 1], 1.0)
        xhT = rp.tile([FD + 1, L], BF16)
        HC = min(1024, L)
        h1g = [rp.tile([H + 1, HC], BF16, name=f"h1g{i}", tag=f"h1g{i}") for i in range(2)]
        nc.gpsimd.memset(h1g[0][H:H + 1, :], 1.0)
        nc.gpsimd.memset(h1g[1][H:H + 1, :], 1.0)
        prodf = rp.tile([P, NB, NF], BF16)
        tmp4 = [rp.tile([P, NB, 4], F32, name=f"tmp4{i}", tag=f"tmp4{i}")
                for i in range(NBUF)]
        vf_b = rp.tile([P, NB], BF16)
        lv_b = rp.tile([P, NB], BF16)
        fid_b = rp.tile([P, NB], BF16)
        gid_b = rp.tile([P, NB], BF16)
        vg = rp.tile([P, NB], BF16)
        vfg = rp.tile([P, NB], BF16)
        lvg = rp.tile([P, NB], BF16)
        cntr = rp.tile([P, NB], F32)
        mus = rp.tile([P, NB], F32)
        sqs = rp.tile([P, NB], F32)
        mu = rp.tile([P, NB], F32)
        varv_t = rp.tile([P, NB], F32)
        rstd = rp.tile([P, NB], F32)
        totT_sb = rp.tile([C, NB], BF16)
        tot_pm = rp.tile([NB, C], BF16)

        def stage_feat(r, b, b0, b1):
            """Static features + scan-input channels for blocks [b0, b1)."""
            NBh = b1 - b0
            tokv = gv[:, r, b0:b1, :]
            varv = gv[:, r, NB + b0:NB + b1, :]
            Sb = S[b][:, b0:b1]
            gohb = goh[b][:, b0:b1]
            famb = famoh[b][:, b0:b1]
            featb = feat[b][:, b0:b1]
            posnh = posn[:, b0:b1]
            vfh, lvh = vf_b[:, b0:b1], lv_b[:, b0:b1]
            fidh, gidh = fid_b[:, b0:b1], gid_b[:, b0:b1]
            vgh, vfgh, lvgh = vg[:, b0:b1], vfg[:, b0:b1], lvg[:, b0:b1]

            nc.gpsimd.tensor_copy(out=featb[:, :, 0:6], in_=tokv[:, :, 0:6])
            nc.gpsimd.tensor_copy(out=featb[:, :, 22:26], in_=varv[:, :, 0:4])
            nc.gpsimd.tensor_copy(out=featb[:, :, 26:27],
                                  in_=posnh[:, :, None])
            # prev_tok_log / prev_tok_signed: shift by one position via PE
            prevp = pmisc.tile([P, NBh, 2], F32, tag="pbank")
            nc.tensor.matmul(out=prevp[:, :, :], lhsT=shift1[:, :],
                             rhs=tokv[:, :, 1:3], start=True, stop=False,
                             skip_group_check=True)
            if b0 > 0:
                nc.tensor.matmul(out=prevp[:, 0:1, :], lhsT=e2m[:, :],
                                 rhs=gv[:, r, b0 - 1, 1:3], start=False,
                                 stop=False, skip_group_check=True)
            nc.tensor.matmul(out=prevp[:, 1:NBh, :], lhsT=e2m[:, :],
                             rhs=tokv[:, 0:NBh - 1, 1:3], start=False,
                             stop=True, skip_group_check=True)
            nc.scalar.copy(out=featb[:, :, 35:37], in_=prevp[:, :, :])

            # bf16 casts of scan drivers
            nc.vector.tensor_copy(out=vfh[:, :], in_=tokv[:, :, 6])
            nc.vector.tensor_tensor(out=lvh[:, :], in0=tokv[:, :, 1],
                                    in1=tokv[:, :, 6], op=OP.mult)
            nc.scalar.copy(out=fidh[:, :], in_=varv[:, :, 4])
            nc.scalar.copy(out=gidh[:, :], in_=varv[:, :, 5])

            nc.scalar.copy(out=Sb[:, :, SC_VF], in_=vfh[:, :])
            nc.scalar.copy(out=Sb[:, :, SC_LV], in_=lvh[:, :])
            nc.vector.tensor_tensor(
                out=famb[:, :, :],
                in0=fidh[:, :, None].broadcast_to([P, NBh, NF]),
                in1=iotf[:, None, :].broadcast_to([P, NBh, NF]),
                op=OP.is_equal)
            nc.scalar.copy(out=featb[:, :, 6:22], in_=famb[:, :, :])
            nc.vector.tensor_tensor(
                out=Sb[:, :, SC_FV:SC_FV + NF], in0=famb[:, :, :],
                in1=vfh[:, :, None].broadcast_to([P, NBh, NF]), op=OP.mult)
            nc.vector.tensor_tensor(
                out=Sb[:, :, SC_FL:SC_FL + NF], in0=famb[:, :, :],
                in1=lvh[:, :, None].broadcast_to([P, NBh, NF]), op=OP.mult)
            nc.vector.tensor_tensor(
                out=gohb[:, :, :],
                in0=gidh[:, :, None].broadcast_to([P, NBh, NG]),
                in1=iotg[:, None, :].broadcast_to([P, NBh, NG]),
                op=OP.is_equal)
            nc.vector.tensor_scalar(out=vgh[:, :], in0=gidh[:, :],
                                    scalar1=0.0, scalar2=None, op0=OP.is_gt)
            nc.vector.tensor_tensor(out=vfgh[:, :], in0=vgh[:, :],
                                    in1=vfh[:, :], op=OP.mult)
            nc.vector.tensor_tensor(out=lvgh[:, :], in0=vgh[:, :],
                                    in1=lvh[:, :], op=OP.mult)
            nc.vector.tensor_tensor(
                out=Sb[:, :, SC_GV:SC_GV + NG], in0=gohb[:, :, :],
                in1=vfgh[:, :, None].broadcast_to([P, NBh, NG]), op=OP.mult)
            nc.vector.tensor_tensor(
                out=Sb[:, :, SC_GL:SC_GL + NG], in0=gohb[:, :, :],
                in1=lvgh[:, :, None].broadcast_to([P, NBh, NG]), op=OP.mult)

        def stage_scan(r, b):
            """Block totals + hierarchical exclusive scan into scan_sb."""
            Sb = S[b]
            totT_ps = ptot.tile([C, NB], F32, tag="tot")
            for blk in range(NB):
                nc.tensor.matmul(out=totT_ps[:, blk:blk + 1],
                                 lhsT=Sb[:, blk, :], rhs=ones_col[:, :],
                                 start=True, stop=True)
            nc.vector.tensor_copy(out=totT_sb[:, :], in_=totT_ps[:, :])
            tot_ps = ptot.tile([NB, C], BF16, tag="tot")
            nc.tensor.transpose(out=tot_ps[:, :], in_=totT_sb[:, :],
                                identity=iden[0:C, 0:C])
            nc.scalar.copy(out=tot_pm[:, :], in_=tot_ps[:, :])

            G = min(4, NB)
            for g in range(NB // G):
                rhs_tmp = iop.tile([NB, G, C], BF16, tag="rhstmp")
                nc.vector.tensor_tensor(
                    out=rhs_tmp[:, :, :],
                    in0=tri64[:, g * G:(g + 1) * G, None].broadcast_to(
                        [NB, G, C]),
                    in1=tot_pm[:, None, :].broadcast_to([NB, G, C]),
                    op=OP.mult)
                scan_ps = pmisc.tile([P, G, C], F32, tag="pbank")
                nc.tensor.matmul(out=scan_ps[:, :, :], lhsT=ones_nb[:, :],
                                 rhs=rhs_tmp[:, :, :], start=True, stop=False)
                for j in range(G):
                    blk = g * G + j
                    nc.tensor.matmul(out=scan_ps[:, j, :], lhsT=tri128[:, :],
                                     rhs=Sb[:, blk, :], start=False,
                                     stop=(j == G - 1))
                dst = scan_sb[b][:, g * G:(g + 1) * G, :]
                if g % 2 == 0:
                    nc.scalar.copy(out=dst, in_=scan_ps[:, :, :])
                else:
                    nc.vector.tensor_copy(out=dst, in_=scan_ps[:, :, :])

        def stage_select(r, b):
            """Select own family/group running stats -> tmp4."""
            scb, t4 = scan_sb[b], tmp4[b]
            nc.vector.tensor_tensor(out=prodf[:, :, :],
                                    in0=scb[:, :, SC_FV:SC_FV + NF],
                                    in1=famoh[b][:, :, :], op=OP.mult)
            nc.vector.tensor_reduce(out=t4[:, :, 0:1], in_=prodf[:, :, :],
                                    axis=mybir.AxisListType.X, op=OP.add)
            nc.vector.tensor_tensor(out=prodf[:, :, :],
                                    in0=scb[:, :, SC_FL:SC_FL + NF],
                                    in1=famoh[b][:, :, :], op=OP.mult)
            nc.vector.tensor_reduce(out=t4[:, :, 1:2], in_=prodf[:, :, :],
                                    axis=mybir.AxisListType.X, op=OP.add)
            # group-sized selects on gpsimd (idle after the gathers)
            prodg = pgp.tile([P, NB, NG], BF16, tag="prodg")
            nc.gpsimd.tensor_tensor(out=prodg[:, :, :],
                                    in0=scb[:, :, SC_GV:SC_GV + NG],
                                    in1=goh[b][:, :, :], op=OP.mult)
            nc.vector.tensor_reduce(out=t4[:, :, 2:3], in_=prodg[:, :, :],
                                    axis=mybir.AxisListType.X, op=OP.add)
            prodg2 = pgp.tile([P, NB, NG], BF16, tag="prodg")
            nc.gpsimd.tensor_tensor(out=prodg2[:, :, :],
                                    in0=scb[:, :, SC_GL:SC_GL + NG],
                                    in1=goh[b][:, :, :], op=OP.mult)
            nc.vector.tensor_reduce(out=t4[:, :, 3:4], in_=prodg2[:, :, :],
                                    axis=mybir.AxisListType.X, op=OP.add)

        def stage_ln(r, b):
            """Dynamic features + LayerNorm + bf16 staging for transpose."""
            scb, featb, t4 = scan_sb[b], feat[b], tmp4[b]
            nc.vector.tensor_scalar(out=featb[:, :, 27:29],
                                    in0=scb[:, :, 0:2],
                                    scalar1=1.0 / denom, scalar2=None,
                                    op0=OP.mult)
            nc.vector.tensor_scalar(out=featb[:, :, 29:33], in0=t4[:, :, :],
                                    scalar1=1.0 / denom, scalar2=None,
                                    op0=OP.mult)
            nc.vector.tensor_scalar(out=cntr[:, :], in0=scb[:, :, SC_VF],
                                    scalar1=1.0, scalar2=None, op0=OP.max)
            nc.vector.reciprocal(out=cntr[:, :], in_=cntr[:, :])
            nc.vector.tensor_tensor(out=featb[:, :, 33], in0=t4[:, :, 0],
                                    in1=cntr[:, :], op=OP.mult)
            nc.vector.tensor_tensor(out=featb[:, :, 34], in0=t4[:, :, 2],
                                    in1=cntr[:, :], op=OP.mult)

            nc.vector.tensor_reduce(out=mus[:, :, None], in_=featb[:, :, :],
                                    axis=mybir.AxisListType.X, op=OP.add)
            nc.scalar.activation(out=sqf[:, :, :], in_=featb[:, :, :],
                                 func=ACTF.Square)
            nc.vector.tensor_reduce(out=sqs[:, :, None], in_=sqf[:, :, :],
                                    axis=mybir.AxisListType.X, op=OP.add)
            nc.vector.tensor_scalar(out=mu[:, :], in0=mus[:, :],
                                    scalar1=1.0 / FD, scalar2=None,
                                    op0=OP.mult)
            nc.vector.tensor_scalar(out=varv_t[:, :], in0=sqs[:, :],
                                    scalar1=1.0 / FD, scalar2=None,
                                    op0=OP.mult)
            nc.vector.tensor_tensor(out=rstd[:, :], in0=mu[:, :],
                                    in1=mu[:, :], op=OP.mult)
            nc.vector.tensor_tensor(out=varv_t[:, :], in0=varv_t[:, :],
                                    in1=rstd[:, :], op=OP.subtract)
            nc.scalar.activation(out=varv_t[:, :], in_=varv_t[:, :],
                                 func=ACTF.Sqrt, bias=eps_t[:, :])
            nc.vector.reciprocal(out=rstd[:, :], in_=varv_t[:, :])
            nc.vector.tensor_tensor(
                out=featb[:, :, :], in0=featb[:, :, :],
                in1=mu[:, :, None].broadcast_to([P, NB, FD]), op=OP.subtract)
            nc.vector.tensor_tensor(
                out=featb[:, :, :], in0=featb[:, :, :],
                in1=rstd[:, :, None].broadcast_to([P, NB, FD]), op=OP.mult)
            nc.vector.tensor_copy(out=xhb[b][:, :, 0:FD],
                                  in_=featb[:, :, :])

        def stage_mlp(r, b):
            """Transpose to channel-major, MLP1+GELU, MLP2, store."""
            for blk in range(NB):
                trp = pmisc.tile([FD + 1, P], BF16, tag="pbank")
                nc.tensor.transpose(out=trp[:, :], in_=xhb[b][:, blk, :],
                                    identity=iden[:, :])
                dst = xhT[:, blk * P:(blk + 1) * P]
                if blk % 2 == 0:
                    nc.scalar.copy(out=dst, in_=trp[:, :])
                else:
                    nc.vector.tensor_copy(out=dst, in_=trp[:, :])

            MT = 512
            NCH = HC // MT           # mm1 tiles per h1g chunk
            NA = 448                 # evacuation split scalar/vector
            for ch in range(L // HC):
                hg = h1g[ch % 2]
                for t in range(NCH):
                    t0 = ch * HC + t * MT
                    h1ps = pmisc.tile([H, MT], F32, tag="pbank")
                    nc.tensor.matmul(out=h1ps[:, :], lhsT=w1e[:, :],
                                     rhs=xhT[:, t0:t0 + MT],
                                     start=True, stop=True)
                    nc.scalar.activation(out=hg[0:H, t * MT:(t + 1) * MT],
                                         in_=h1ps[:, :], func=ACTF.Gelu)
                for bb in range(HC // P):
                    blk = ch * (HC // P) + bb
                    lhs = hg[:, bb * P:(bb + 1) * P]
                    ps_a = pmm2.tile([P, NA], F32, tag="mm2")
                    nc.tensor.matmul(out=ps_a[:, :], lhsT=lhs,
                                     rhs=w2e[:, 0:NA], start=True, stop=True)
                    ps_b = pmm2.tile([P, D - NA], F32, tag="mm2")
                    nc.tensor.matmul(out=ps_b[:, :], lhsT=lhs,
                                     rhs=w2e[:, NA:D], start=True, stop=True)
                    osb = op_.tile([P, D], BF16, tag="osb")
                    nc.scalar.copy(out=osb[:, 0:NA], in_=ps_a[:, :])
                    nc.vector.tensor_copy(out=osb[:, NA:D], in_=ps_b[:, :])
                    nc.sync.dma_start(
                        out=out_d[r, blk * P:(blk + 1) * P, :],
                        in_=osb[:, :])

        # ---- interleaved two-row schedule ----
        # MLPs last: their PSUM-evacuation streams pipeline across rows
        # without blocking the other row's vector-engine stages.
        NH = NB // 2
        if R == 1:
            stage_feat(0, 0, 0, NH)
            stage_feat(0, 0, NH, NB)
            stage_scan(0, 0)
            stage_select(0, 0)
            stage_ln(0, 0)
            stage_mlp(0, 0)
        else:
            for rr in range(0, R, 2):
                stage_feat(rr, 0, 0, NH)
                stage_feat(rr, 0, NH, NB)
                stage_scan(rr, 0)
                stage_feat(rr + 1, 1, 0, NH)
                stage_feat(rr + 1, 1, NH, NB)
                stage_select(rr, 0)
                stage_ln(rr, 0)
                stage_scan(rr + 1, 1)
                stage_select(rr + 1, 1)
                stage_ln(rr + 1, 1)
                stage_mlp(rr, 0)
                stage_mlp(rr + 1, 1)


def build_program(R, L):
    nc = bacc.Bacc("TRN2", target_bir_lowering=False, debug=False,
                   num_swdge_queues=NQ)
    NB = L // P
    NIQ = 2 * L // NQ
    ins = {
        "gidx": nc.dram_tensor("gidx", [P, R, NIQ // 16], I16,
                               kind="ExternalInput").ap(),
        "gtable": nc.dram_tensor("gtable", [V + NV, 64], F32,
                                 kind="ExternalInput").ap(),
        "w1e": nc.dram_tensor("w1e", [FD + 1, H], BF16,
                              kind="ExternalInput").ap(),
        "w2e": nc.dram_tensor("w2e", [H + 1, D], BF16,
                              kind="ExternalInput").ap(),
        "cst_f32": nc.dram_tensor("cst_f32", [P, P + P + NB + 1], F32,
                                  kind="ExternalInput").ap(),
        "cst_b16": nc.dram_tensor("cst_b16",
                                  [P, P + P + NB + NF + NG + 1 + P], BF16,
                                  kind="ExternalInput").ap(),
    }
    outs = {
        "out": nc.dram_tensor("out", [R, L, D], BF16,
                              kind="ExternalOutput").ap(),
    }
    with tile.TileContext(nc) as tc:
        emit(tc, ins, outs, R, L)
    nc.compile()
    return nc


def make_consts(L):
    """Host-precomputed constant matrices, packed along the free dim."""
    NB = L // P
    denom = float(max(L - 1, 1))
    bf = ml_dtypes.bfloat16
    # f32 pack: shift1, e2m, posn, eps
    # shift1[s, p] = 1 iff p == s + 1
    shift1 = np.zeros((P, P), np.float32)
    shift1[np.arange(P - 1), np.arange(P - 1) + 1] = 1.0
    e2m = np.zeros((P, P), np.float32)
    e2m[P - 1, 0] = 1.0
    posn = (np.arange(NB)[None, :] * P
            + np.arange(P)[:, None]).astype(np.float32) / denom
    eps = np.full((P, 1), EPS, np.float32)
    cst_f32 = np.concatenate([shift1, e2m, posn, eps], axis=1)
    # bf16 pack: tri128, iden, tri64, iotf, iotg, ones_col, ones_nb
    tri128 = np.triu(np.ones((P, P), np.float32), 1)
    iden = np.eye(P, dtype=np.float32)
    tri64 = np.zeros((P, NB), np.float32)
    tri64[0:NB] = np.triu(np.ones((NB, NB), np.float32), 1)
    iotf = np.broadcast_to(np.arange(NF, dtype=np.float32), (P, NF))
    iotg = np.broadcast_to(np.arange(NG, dtype=np.float32), (P, NG))
    onec = np.ones((P, 1), np.float32)
    onen = np.zeros((P, P), np.float32)
    onen[0:NB] = 1.0
    cst_b16 = np.concatenate([tri128, iden, tri64, iotf, iotg, onec, onen],
                             axis=1).astype(bf)
    return cst_f32, cst_b16


def prep_host(inputs, n_cores, R, L):
    """Pack tables/weights, shard+transpose indices. Returns in_maps list."""
    f32 = np.float32
    tok_ids = np.asarray(inputs["token_ids"])
    var_ids = np.asarray(inputs["var_ids"])

    has_int = np.asarray(inputs["token_has_int"], f32)
    vmask = np.ones(has_int.shape[0], f32)
    vmask[[0, 1, 2]] = 0.0
    validf = (has_int > 0).astype(f32) * vmask
    fam_id = np.asarray(inputs["var_family_id"], f32)
    gtable = np.zeros((V + NV, 64), f32)
    gtable[:V, 0] = has_int
    gtable[:V, 1] = np.asarray(inputs["token_log_norm"], f32)
    gtable[:V, 2] = np.asarray(inputs["token_signed_norm"], f32)
    gtable[:V, 3] = np.asarray(inputs["token_is_zero"], f32)
    gtable[:V, 4] = np.asarray(inputs["token_is_one"], f32)
    gtable[:V, 5] = np.asarray(inputs["token_is_pow2"], f32)
    gtable[:V, 6] = validf
    gtable[V:, 0] = np.asarray(inputs["var_outer_norm"], f32)
    gtable[V:, 1] = np.asarray(inputs["var_inner_norm"], f32)
    gtable[V:, 2] = np.asarray(inputs["var_has_outer"], f32)
    gtable[V:, 3] = np.asarray(inputs["var_has_inner"], f32)
    gtable[V:, 4] = fam_id
    gtable[V:, 5] = np.asarray(inputs["var_group_id"], f32)

    W1 = np.asarray(inputs["W1"], f32)
    b1 = np.asarray(inputs["b1"], f32)
    W2 = np.asarray(inputs["W2"], f32)
    b2 = np.asarray(inputs["b2"], f32)
    gamma = np.asarray(inputs["ln_gamma"], f32)
    beta = np.asarray(inputs["ln_beta"], f32)
    scale = np.float32(np.asarray(inputs["scale"]))

    W1g = gamma[:, None] * W1
    w1e = np.concatenate([W1g[REF_PERM], (beta @ W1 + b1)[None]],
                         axis=0).astype(ml_dtypes.bfloat16)
    w2e = np.concatenate([W2 * scale, (b2 * scale)[None]],
                         axis=0).astype(ml_dtypes.bfloat16)

    cst_f32, cst_b16 = make_consts(L)

    in_maps = []
    NIQ = 2 * L // NQ
    cols = NIQ // 16
    ar = np.arange(NIQ)
    for c in range(n_cores):
        gidx = np.zeros((P, R, cols), np.int16)
        for r in range(R):
            flat = np.concatenate([
                tok_ids[c * R + r],
                var_ids[c * R + r].astype(np.int64) + V,
            ]).astype(np.int16)
            for q in range(NQ):
                chunk = flat[q * NIQ:(q + 1) * NIQ]
                w16 = np.zeros((16, cols), np.int16)
                w16[ar % 16, ar // 16] = chunk
                gidx[32 * q:32 * q + 16, r] = w16
                gidx[32 * q + 16:32 * q + 32, r] = w16
        in_maps.append({
            "gidx": gidx,
            "gtable": gtable,
            "w1e": w1e,
            "w2e": w2e,
            "cst_f32": cst_f32,
            "cst_b16": cst_b16,
        })
    return in_maps


_CACHE = {}


def _get_program(R, L):
    key = (R, L)
    if key not in _CACHE:
        _CACHE[key] = build_program(R, L)
    return _CACHE[key]


def kernel(**inputs):
    from concourse.bass_utils import run_bass_kernel_spmd

    B, L = np.asarray(inputs["token_ids"]).shape
    n_cores = 8
    R = B // n_cores
    nc = _get_program(R, L)
    in_maps = prep_host(inputs, n_cores, R, L)
    trace = bool(int(os.environ.get("KERNEL_TRACE", "0")))
    try:
        res = run_bass_kernel_spmd(nc, in_maps,
                                   core_ids=list(range(n_cores)),
                                   trace=trace)
    except Exception:
        if not trace:
            raise
        res = run_bass_kernel_spmd(nc, in_maps,
                                   core_ids=list(range(n_cores)),
                                   trace=False)
    kernel.last_results = res
    out = np.concatenate([np.asarray(r["out"]) for r in res.results], axis=0)
    return out.astype(np.float32)


# revision 17
# speedup vs baseline: 1.2872x; 1.2872x over previous
"""Trainium2 Bass kernel for nn_DecoderDynamicTokenSideEmbedding.

Data-parallel across 8 NeuronCores: each core processes 2 batch rows of
[8192] tokens. Full inputs in, full [16, 8192, 768] f32 output back.

Per-core structure (pos-major layout: partition = position-within-block,
free dim = block index, 64 blocks of 128 positions per row):
  1. all constant matrices are host-precomputed and DMA'd in, so gpsimd
     only ever runs the mlp ucode library (no IRAM thrash)
  2. indirect-DMA gathers from the packed [V+NV, 64] f32 table, split
     across the 4 SWDGE queues so all four Q7 core pairs generate
     descriptors concurrently; both rows issued up front
  3. exclusive prefix sums via strict-upper-triangular bf16 matmuls
     (per-block [128x128] + hierarchical block-carry)
  4. family/group running stats selected by one-hot multiply + reduce
     (group-sized ops offloaded to gpsimd, which is idle after gathers)
  5. LayerNorm in f32 (gamma/beta folded into W1 on host)
  6. PE transpose to channel-major bf16, then MLP:
     [38,64] matmul -> exact GELU -> [65,768] matmul (b1/b2/scale folded)
  7. output written to HBM as bf16 (2 blocks per DMA), widened on host

Rows are double-buffered (S/scan/one-hots/feat) and their stages
interleaved so row 1's vector work fills row 0's PE/evacuation phases.
"""

import os

import numpy as np
import ml_dtypes

import concourse.bass as bass
import concourse.bacc as bacc
import concourse.tile as tile
import concourse.mybir as mybir
from concourse.library_config import mlp as _mlp_lib

F32 = mybir.dt.float32
BF16 = mybir.dt.bfloat16
I32 = mybir.dt.int32
I16 = mybir.dt.int16
OP = mybir.AluOpType
ACTF = mybir.ActivationFunctionType

P = 128
NF, NG = 16, 32
C = 2 + 2 * NF + 2 * NG  # 98 scan channels
FD = 37                  # true feature count
H, D = 64, 768
V, NV = 32000, 512
EPS = 1e-5
NQ = 4                   # SWDGE queues used for gathers

# scan channel layout
SC_VF, SC_LV = 0, 1
SC_FV, SC_FL = 2, 2 + NF            # fam_valid [2:18), fam_logv [18:34)
SC_GV, SC_GL = 2 + 2 * NF, 2 + 2 * NF + NG  # [34:66), [66:98)

# packed host-constant layouts (free-dim offsets)
#   cst_f32 [P, 128+128+NB+1]: shift1, e2m, posn, eps
#   cst_b16 [P, 128+128+NB+NF+NG+1+128]: tri128, iden, tri64, iotf, iotg,
#                                        ones_col, ones_nb

# device feature layout (feat tile channel index)
# 0 has_int, 1 tok_log, 2 tok_signed, 3 zero, 4 one, 5 pow2,
# 6:22 fam_oh, 22 outer, 23 inner, 24 has_outer, 25 has_inner,
# 26 pos_norm, 27 prev_count_n, 28 prev_logsum_n,
# 29 psf_c_n, 30 psf_l_n, 31 psg_c_n, 32 psg_l_n,
# 33 psf_ratio, 34 psg_ratio, 35 prev_tok_log, 36 prev_tok_signed
REF_PERM = ([0, 2, 1, 3, 4, 5] + list(range(6, 22)) + [22, 23, 24, 25]
            + [26, 27, 28, 29, 30, 32, 33, 31, 34, 35, 36])


def emit(tc, ins, outs, R, L):
    """Emit the per-core program. ins/outs: dicts of DRAM APs."""
    nc = tc.nc
    NB = L // P
    denom = float(max(L - 1, 1))
    gidx_d, gtable = ins["gidx"], ins["gtable"]
    w1e_d, w2e_d = ins["w1e"], ins["w2e"]
    cf_d, cb_d = ins["cst_f32"], ins["cst_b16"]
    out_d = outs["out"]
    NI = 2 * L
    NIQ = NI // NQ       # indices per gather-queue chunk
    BC = NIQ // P        # gv columns per chunk
    NBUF = min(2, R)

    with (
        tc.tile_pool(name="const", bufs=1) as cp,
        tc.tile_pool(name="row", bufs=1) as rp,
        tc.tile_pool(name="io", bufs=2) as iop,
        tc.tile_pool(name="pg", bufs=1) as pgp,
        tc.tile_pool(name="osb", bufs=4) as op_,
        tc.tile_pool(name="pmisc", bufs=3, space="PSUM") as pmisc,
        tc.tile_pool(name="ptot", bufs=1, space="PSUM") as ptot,
        tc.tile_pool(name="pmm2", bufs=4, space="PSUM") as pmm2,
    ):
        nc.gpsimd.load_library(_mlp_lib)

        # ---- input DMAs ----
        gix = cp.tile([P, R, NIQ // 16], I16)
        nc.sync.dma_start(out=gix[:, :, :], in_=gidx_d[:, :, :])
        w1e = cp.tile([FD + 1, H], BF16)
        nc.sync.dma_start(out=w1e[:, :], in_=w1e_d[:, :])
        w2e = cp.tile([H + 1, D], BF16)
        nc.sync.dma_start(out=w2e[:, :], in_=w2e_d[:, :])
        FTOT = P + P + NB + 1
        cst_f = cp.tile([P, FTOT], F32)
        nc.sync.dma_start(out=cst_f[:, :], in_=cf_d[:, :])
        BTOT = P + P + NB + NF + NG + 1 + P
        cst_b = cp.tile([P, BTOT], BF16)
        nc.sync.dma_start(out=cst_b[:, :], in_=cb_d[:, :])

        shift1 = cst_f[:, 0:P]
        e2m = cst_f[:, P:2 * P]
        posn = cst_f[:, 2 * P:2 * P + NB]
        eps_t = cst_f[:, 2 * P + NB:2 * P + NB + 1]
        o = 0
        tri128 = cst_b[:, o:o + P]; o += P
        iden = cst_b[:, o:o + P]; o += P
        tri64 = cst_b[0:NB, o:o + NB]; o += NB
        iotf = cst_b[:, o:o + NF]; o += NF
        iotg = cst_b[:, o:o + NG]; o += NG
        ones_col = cst_b[:, o:o + 1]; o += 1
        ones_nb = cst_b[0:NB, o:o + P]; o += P

        # ---- all gathers up front: 4 SWDGE queues x R rows ----
        gv = rp.tile([P, R, 2 * NB, 64], F32)
        for r in range(R):
            for q in range(NQ):
                nc.gpsimd.dma_gather(
                    gv[:, r, q * BC:(q + 1) * BC, :], gtable[:, :],
                    gix[:, r, :], NIQ, NIQ, 64,
                    single_packet=False, queue_num=q)

        # ---- persistent row tiles (double-buffered where rows overlap) ----
        S = [rp.tile([P, NB, C], BF16, name=f"S{i}", tag=f"S{i}") for i in range(NBUF)]
        goh = [rp.tile([P, NB, NG], BF16, name=f"goh{i}", tag=f"goh{i}")
               for i in range(NBUF)]
        famoh = [rp.tile([P, NB, NF], BF16, name=f"famoh{i}", tag=f"famoh{i}")
                 for i in range(NBUF)]
        scan_sb = [rp.tile([P, NB, C], BF16, name=f"scan{i}", tag=f"scan{i}")
                   for i in range(NBUF)]
        feat = [rp.tile([P, NB, FD], F32, name=f"feat{i}", tag=f"feat{i}")
                for i in range(NBUF)]
        sqf = rp.tile([P, NB, FD], F32)
        xhb = [rp.tile([P, NB, FD + 1], BF16, name=f"xhb{i}", tag=f"xhb{i}")
               for i in range(NBUF)]
        for i in range(NBUF):
            nc.gpsimd.memset(xhb[i][:, :, FD:FD + 1], 1.0)
        xhT = rp.tile([FD + 1, L], BF16)
        HC = min(1024, L)
        h1g = [rp.tile([H + 1, HC], BF16, name=f"h1g{i}", tag=f"h1g{i}") for i in range(2)]
        nc.gpsimd.memset(h1g[0][H:H + 1, :], 1.0)
        nc.gpsimd.memset(h1g[1][H:H + 1, :], 1.0)
        prodf = rp.tile([P, NB, NF], BF16)
        tmp4 = [rp.tile([P, NB, 4], F32, name=f"tmp4{i}", tag=f"tmp4{i}")
                for i in range(NBUF)]
        vf_b = rp.tile([P, NB], BF16)
        lv_b = rp.tile([P, NB], BF16)
        fid_b = rp.tile([P, NB], BF16)
        gid_b = rp.tile([P, NB], BF16)
        vg = rp.tile([P, NB], BF16)
        vfg = rp.tile([P, NB], BF16)
        lvg = rp.tile([P, NB], BF16)
        cntr = rp.tile([P, NB], F32)
        mus = rp.tile([P, NB], F32)
        sqs = rp.tile([P, NB], F32)
        mu = rp.tile([P, NB], F32)
        varv_t = rp.tile([P, NB], F32)
        rstd = rp.tile([P, NB], F32)
        totT_sb = rp.tile([C, NB], BF16)
        tot_pm = rp.tile([NB, C], BF16)

        def stage_feat(r, b, b0, b1):
            """Static features + scan-input channels for blocks [b0, b1)."""
            NBh = b1 - b0
            tokv = gv[:, r, b0:b1, :]
            varv = gv[:, r, NB + b0:NB + b1, :]
            Sb = S[b][:, b0:b1]
            gohb = goh[b][:, b0:b1]
            famb = famoh[b][:, b0:b1]
            featb = feat[b][:, b0:b1]
            posnh = posn[:, b0:b1]
            vfh, lvh = vf_b[:, b0:b1], lv_b[:, b0:b1]
            fidh, gidh = fid_b[:, b0:b1], gid_b[:, b0:b1]
            vgh, vfgh, lvgh = vg[:, b0:b1], vfg[:, b0:b1], lvg[:, b0:b1]

            nc.gpsimd.tensor_copy(out=featb[:, :, 0:6], in_=tokv[:, :, 0:6])
            nc.gpsimd.tensor_copy(out=featb[:, :, 22:26], in_=varv[:, :, 0:4])
            nc.gpsimd.tensor_copy(out=featb[:, :, 26:27],
                                  in_=posnh[:, :, None])
            # prev_tok_log / prev_tok_signed: shift by one position via PE
            prevp = pmisc.tile([P, NBh, 2], F32, tag="pbank")
            nc.tensor.matmul(out=prevp[:, :, :], lhsT=shift1[:, :],
                             rhs=tokv[:, :, 1:3], start=True, stop=False,
                             skip_group_check=True)
            if b0 > 0:
                nc.tensor.matmul(out=prevp[:, 0:1, :], lhsT=e2m[:, :],
                                 rhs=gv[:, r, b0 - 1, 1:3], start=False,
                                 stop=False, skip_group_check=True)
            nc.tensor.matmul(out=prevp[:, 1:NBh, :], lhsT=e2m[:, :],
                             rhs=tokv[:, 0:NBh - 1, 1:3], start=False,
                             stop=True, skip_group_check=True)
            nc.scalar.copy(out=featb[:, :, 35:37], in_=prevp[:, :, :])

            # bf16 casts of scan drivers
            nc.vector.tensor_copy(out=vfh[:, :], in_=tokv[:, :, 6])
            nc.vector.tensor_tensor(out=lvh[:, :], in0=tokv[:, :, 1],
                                    in1=tokv[:, :, 6], op=OP.mult)
            nc.scalar.copy(out=fidh[:, :], in_=varv[:, :, 4])
            nc.scalar.copy(out=gidh[:, :], in_=varv[:, :, 5])

            nc.scalar.copy(out=Sb[:, :, SC_VF], in_=vfh[:, :])
            nc.scalar.copy(out=Sb[:, :, SC_LV], in_=lvh[:, :])
            nc.vector.tensor_tensor(
                out=famb[:, :, :],
                in0=fidh[:, :, None].broadcast_to([P, NBh, NF]),
                in1=iotf[:, None, :].broadcast_to([P, NBh, NF]),
                op=OP.is_equal)
            nc.scalar.copy(out=featb[:, :, 6:22], in_=famb[:, :, :])
            nc.vector.tensor_tensor(
                out=Sb[:, :, SC_FV:SC_FV + NF], in0=famb[:, :, :],
                in1=vfh[:, :, None].broadcast_to([P, NBh, NF]), op=OP.mult)
            nc.vector.tensor_tensor(
                out=Sb[:, :, SC_FL:SC_FL + NF], in0=famb[:, :, :],
                in1=lvh[:, :, None].broadcast_to([P, NBh, NF]), op=OP.mult)
            nc.vector.tensor_tensor(
                out=gohb[:, :, :],
                in0=gidh[:, :, None].broadcast_to([P, NBh, NG]),
                in1=iotg[:, None, :].broadcast_to([P, NBh, NG]),
                op=OP.is_equal)
            nc.vector.tensor_scalar(out=vgh[:, :], in0=gidh[:, :],
                                    scalar1=0.0, scalar2=None, op0=OP.is_gt)
            nc.vector.tensor_tensor(out=vfgh[:, :], in0=vgh[:, :],
                                    in1=vfh[:, :], op=OP.mult)
            nc.vector.tensor_tensor(out=lvgh[:, :], in0=vgh[:, :],
                                    in1=lvh[:, :], op=OP.mult)
            nc.vector.tensor_tensor(
                out=Sb[:, :, SC_GV:SC_GV + NG], in0=gohb[:, :, :],
                in1=vfgh[:, :, None].broadcast_to([P, NBh, NG]), op=OP.mult)
            nc.vector.tensor_tensor(
                out=Sb[:, :, SC_GL:SC_GL + NG], in0=gohb[:, :, :],
                in1=lvgh[:, :, None].broadcast_to([P, NBh, NG]), op=OP.mult)

        def stage_scan(r, b):
            """Block totals + hierarchical exclusive scan into scan_sb."""
            Sb = S[b]
            totT_ps = ptot.tile([C, NB], F32, tag="tot")
            for blk in range(NB):
                nc.tensor.matmul(out=totT_ps[:, blk:blk + 1],
                                 lhsT=Sb[:, blk, :], rhs=ones_col[:, :],
                                 start=True, stop=True)
            nc.vector.tensor_copy(out=totT_sb[:, :], in_=totT_ps[:, :])
            tot_ps = ptot.tile([NB, C], BF16, tag="tot")
            nc.tensor.transpose(out=tot_ps[:, :], in_=totT_sb[:, :],
                                identity=iden[0:C, 0:C])
            nc.scalar.copy(out=tot_pm[:, :], in_=tot_ps[:, :])

            G = min(4, NB)
            for g in range(NB // G):
                rhs_tmp = iop.tile([NB, G, C], BF16, tag="rhstmp")
                nc.vector.tensor_tensor(
                    out=rhs_tmp[:, :, :],
                    in0=tri64[:, g * G:(g + 1) * G, None].broadcast_to(
                        [NB, G, C]),
                    in1=tot_pm[:, None, :].broadcast_to([NB, G, C]),
                    op=OP.mult)
                scan_ps = pmisc.tile([P, G, C], F32, tag="pbank")
                nc.tensor.matmul(out=scan_ps[:, :, :], lhsT=ones_nb[:, :],
                                 rhs=rhs_tmp[:, :, :], start=True, stop=False)
                for j in range(G):
                    blk = g * G + j
                    nc.tensor.matmul(out=scan_ps[:, j, :], lhsT=tri128[:, :],
                                     rhs=Sb[:, blk, :], start=False,
                                     stop=(j == G - 1))
                dst = scan_sb[b][:, g * G:(g + 1) * G, :]
                if g % 2 == 0:
                    nc.scalar.copy(out=dst, in_=scan_ps[:, :, :])
                else:
                    nc.vector.tensor_copy(out=dst, in_=scan_ps[:, :, :])

        def stage_select(r, b):
            """Select own family/group running stats -> tmp4."""
            scb, t4 = scan_sb[b], tmp4[b]
            nc.vector.tensor_tensor(out=prodf[:, :, :],
                                    in0=scb[:, :, SC_FV:SC_FV + NF],
                                    in1=famoh[b][:, :, :], op=OP.mult)
            nc.vector.tensor_reduce(out=t4[:, :, 0:1], in_=prodf[:, :, :],
                                    axis=mybir.AxisListType.X, op=OP.add)
            nc.vector.tensor_tensor(out=prodf[:, :, :],
                                    in0=scb[:, :, SC_FL:SC_FL + NF],
                                    in1=famoh[b][:, :, :], op=OP.mult)
            nc.vector.tensor_reduce(out=t4[:, :, 1:2], in_=prodf[:, :, :],
                                    axis=mybir.AxisListType.X, op=OP.add)
            # group-sized selects on gpsimd (idle after the gathers)
            prodg = pgp.tile([P, NB, NG], BF16, tag="prodg")
            nc.gpsimd.tensor_tensor(out=prodg[:, :, :],
                                    in0=scb[:, :, SC_GV:SC_GV + NG],
                                    in1=goh[b][:, :, :], op=OP.mult)
            nc.vector.tensor_reduce(out=t4[:, :, 2:3], in_=prodg[:, :, :],
                                    axis=mybir.AxisListType.X, op=OP.add)
            prodg2 = pgp.tile([P, NB, NG], BF16, tag="prodg")
            nc.gpsimd.tensor_tensor(out=prodg2[:, :, :],
                                    in0=scb[:, :, SC_GL:SC_GL + NG],
                                    in1=goh[b][:, :, :], op=OP.mult)
            nc.vector.tensor_reduce(out=t4[:, :, 3:4], in_=prodg2[:, :, :],
                                    axis=mybir.AxisListType.X, op=OP.add)

        def stage_ln(r, b):
            """Dynamic features + LayerNorm + bf16 staging for transpose."""
            scb, featb, t4 = scan_sb[b], feat[b], tmp4[b]
            nc.vector.tensor_scalar(out=featb[:, :, 27:29],
                                    in0=scb[:, :, 0:2],
                                    scalar1=1.0 / denom, scalar2=None,
                                    op0=OP.mult)
            nc.vector.tensor_scalar(out=featb[:, :, 29:33], in0=t4[:, :, :],
                                    scalar1=1.0 / denom, scalar2=None,
                                    op0=OP.mult)
            nc.vector.tensor_scalar(out=cntr[:, :], in0=scb[:, :, SC_VF],
                                    scalar1=1.0, scalar2=None, op0=OP.max)
            nc.vector.reciprocal(out=cntr[:, :], in_=cntr[:, :])
            nc.vector.tensor_tensor(out=featb[:, :, 33], in0=t4[:, :, 0],
                                    in1=cntr[:, :], op=OP.mult)
            nc.vector.tensor_tensor(out=featb[:, :, 34], in0=t4[:, :, 2],
                                    in1=cntr[:, :], op=OP.mult)

            nc.vector.tensor_reduce(out=mus[:, :, None], in_=featb[:, :, :],
                                    axis=mybir.AxisListType.X, op=OP.add)
            nc.scalar.activation(out=sqf[:, :, :], in_=featb[:, :, :],
                                 func=ACTF.Square)
            nc.vector.tensor_reduce(out=sqs[:, :, None], in_=sqf[:, :, :],
                                    axis=mybir.AxisListType.X, op=OP.add)
            nc.vector.tensor_scalar(out=mu[:, :], in0=mus[:, :],
                                    scalar1=1.0 / FD, scalar2=None,
                                    op0=OP.mult)
            nc.vector.tensor_scalar(out=varv_t[:, :], in0=sqs[:, :],
                                    scalar1=1.0 / FD, scalar2=None,
                                    op0=OP.mult)
            nc.vector.tensor_tensor(out=rstd[:, :], in0=mu[:, :],
                                    in1=mu[:, :], op=OP.mult)
            nc.vector.tensor_tensor(out=varv_t[:, :], in0=varv_t[:, :],
                                    in1=rstd[:, :], op=OP.subtract)
            nc.scalar.activation(out=varv_t[:, :], in_=varv_t[:, :],
                                 func=ACTF.Sqrt, bias=eps_t[:, :])
            nc.vector.reciprocal(out=rstd[:, :], in_=varv_t[:, :])
            nc.vector.tensor_tensor(
                out=featb[:, :, :], in0=featb[:, :, :],
                in1=mu[:, :, None].broadcast_to([P, NB, FD]), op=OP.subtract)
            nc.vector.tensor_tensor(
                out=featb[:, :, :], in0=featb[:, :, :],
                in1=rstd[:, :, None].broadcast_to([P, NB, FD]), op=OP.mult)
            nc.vector.tensor_copy(out=xhb[b][:, :, 0:FD],
                                  in_=featb[:, :, :])

        def stage_mlp(r, b):
            """Transpose to channel-major, MLP1+GELU, MLP2, store."""
            for blk in range(NB):
                trp = pmisc.tile([FD + 1, P], BF16, tag="pbank")
                nc.tensor.transpose(out=trp[:, :], in_=xhb[b][:, blk, :],
                                    identity=iden[:, :])
                dst = xhT[:, blk * P:(blk + 1) * P]
                if blk % 2 == 0:
                    nc.scalar.copy(out=dst, in_=trp[:, :])
                else:
                    nc.vector.tensor_copy(out=dst, in_=trp[:, :])

            MT = 512
            NCH = HC // MT           # mm1 tiles per h1g chunk
            NA = 512                 # evacuation split scalar/vector
            for ch in range(L // HC):
                hg = h1g[ch % 2]
                for t in range(NCH):
                    t0 = ch * HC + t * MT
                    h1ps = pmisc.tile([H, MT], F32, tag="pbank")
                    nc.tensor.matmul(out=h1ps[:, :], lhsT=w1e[:, :],
                                     rhs=xhT[:, t0:t0 + MT],
                                     start=True, stop=True)
                    nc.scalar.activation(out=hg[0:H, t * MT:(t + 1) * MT],
                                         in_=h1ps[:, :], func=ACTF.Gelu)
                for bb in range(HC // P):
                    blk = ch * (HC // P) + bb
                    lhs = hg[:, bb * P:(bb + 1) * P]
                    ps_a = pmm2.tile([P, NA], F32, tag="mm2")
                    nc.tensor.matmul(out=ps_a[:, :], lhsT=lhs,
                                     rhs=w2e[:, 0:NA], start=True, stop=True)
                    ps_b = pmm2.tile([P, D - NA], F32, tag="mm2")
                    nc.tensor.matmul(out=ps_b[:, :], lhsT=lhs,
                                     rhs=w2e[:, NA:D], start=True, stop=True)
                    osb = op_.tile([P, D], BF16, tag="osb")
                    nc.scalar.copy(out=osb[:, 0:NA], in_=ps_a[:, :])
                    nc.vector.tensor_copy(out=osb[:, NA:D], in_=ps_b[:, :])
                    nc.sync.dma_start(
                        out=out_d[r, blk * P:(blk + 1) * P, :],
                        in_=osb[:, :])

        # ---- interleaved two-row schedule ----
        # MLPs last: their PSUM-evacuation streams pipeline across rows
        # without blocking the other row's vector-engine stages.
        NH = NB // 2
        if R == 1:
            stage_feat(0, 0, 0, NH)
            stage_feat(0, 0, NH, NB)
            stage_scan(0, 0)
            stage_select(0, 0)
            stage_ln(0, 0)
            stage_mlp(0, 0)
        else:
            for rr in range(0, R, 2):
                stage_feat(rr, 0, 0, NH)
                stage_feat(rr, 0, NH, NB)
                stage_scan(rr, 0)
                stage_feat(rr + 1, 1, 0, NH)
                stage_feat(rr + 1, 1, NH, NB)
                stage_select(rr, 0)
                stage_ln(rr, 0)
                stage_scan(rr + 1, 1)
                stage_mlp(rr, 0)
                stage_select(rr + 1, 1)
                stage_ln(rr + 1, 1)
                stage_mlp(rr + 1, 1)


def build_program(R, L):
    nc = bacc.Bacc("TRN2", target_bir_lowering=False, debug=False,
                   num_swdge_queues=NQ)
    NB = L // P
    NIQ = 2 * L // NQ
    ins = {
        "gidx": nc.dram_tensor("gidx", [P, R, NIQ // 16], I16,
                               kind="ExternalInput").ap(),
        "gtable": nc.dram_tensor("gtable", [V + NV, 64], F32,
                                 kind="ExternalInput").ap(),
        "w1e": nc.dram_tensor("w1e", [FD + 1, H], BF16,
                              kind="ExternalInput").ap(),
        "w2e": nc.dram_tensor("w2e", [H + 1, D], BF16,
                              kind="ExternalInput").ap(),
        "cst_f32": nc.dram_tensor("cst_f32", [P, P + P + NB + 1], F32,
                                  kind="ExternalInput").ap(),
        "cst_b16": nc.dram_tensor("cst_b16",
                                  [P, P + P + NB + NF + NG + 1 + P], BF16,
                                  kind="ExternalInput").ap(),
    }
    outs = {
        "out": nc.dram_tensor("out", [R, L, D], BF16,
                              kind="ExternalOutput").ap(),
    }
    with tile.TileContext(nc) as tc:
        emit(tc, ins, outs, R, L)
    nc.compile()
    return nc


def make_consts(L):
    """Host-precomputed constant matrices, packed along the free dim."""
    NB = L // P
    denom = float(max(L - 1, 1))
    bf = ml_dtypes.bfloat16
    # f32 pack: shift1, e2m, posn, eps
    # shift1[s, p] = 1 iff p == s + 1
    shift1 = np.zeros((P, P), np.float32)
    shift1[np.arange(P - 1), np.arange(P - 1) + 1] = 1.0
    e2m = np.zeros((P, P), np.float32)
    e2m[P - 1, 0] = 1.0
    posn = (np.arange(NB)[None, :] * P
            + np.arange(P)[:, None]).astype(np.float32) / denom
    eps = np.full((P, 1), EPS, np.float32)
    cst_f32 = np.concatenate([shift1, e2m, posn, eps], axis=1)
    # bf16 pack: tri128, iden, tri64, iotf, iotg, ones_col, ones_nb
    tri128 = np.triu(np.ones((P, P), np.float32), 1)
    iden = np.eye(P, dtype=np.float32)
    tri64 = np.zeros((P, NB), np.float32)
    tri64[0:NB] = np.triu(np.ones((NB, NB), np.float32), 1)
    iotf = np.broadcast_to(np.arange(NF, dtype=np.float32), (P, NF))
    iotg = np.broadcast_to(np.arange(NG, dtype=np.float32), (P, NG))
    onec = np.ones((P, 1), np.float32)
    onen = np.zeros((P, P), np.float32)
    onen[0:NB] = 1.0
    cst_b16 = np.concatenate([tri128, iden, tri64, iotf, iotg, onec, onen],
                             axis=1).astype(bf)
    return cst_f32, cst_b16


def prep_host(inputs, n_cores, R, L):
    """Pack tables/weights, shard+transpose indices. Returns in_maps list."""
    f32 = np.float32
    tok_ids = np.asarray(inputs["token_ids"])
    var_ids = np.asarray(inputs["var_ids"])

    has_int = np.asarray(inputs["token_has_int"], f32)
    vmask = np.ones(has_int.shape[0], f32)
    vmask[[0, 1, 2]] = 0.0
    validf = (has_int > 0).astype(f32) * vmask
    fam_id = np.asarray(inputs["var_family_id"], f32)
    gtable = np.zeros((V + NV, 64), f32)
    gtable[:V, 0] = has_int
    gtable[:V, 1] = np.asarray(inputs["token_log_norm"], f32)
    gtable[:V, 2] = np.asarray(inputs["token_signed_norm"], f32)
    gtable[:V, 3] = np.asarray(inputs["token_is_zero"], f32)
    gtable[:V, 4] = np.asarray(inputs["token_is_one"], f32)
    gtable[:V, 5] = np.asarray(inputs["token_is_pow2"], f32)
    gtable[:V, 6] = validf
    gtable[V:, 0] = np.asarray(inputs["var_outer_norm"], f32)
    gtable[V:, 1] = np.asarray(inputs["var_inner_norm"], f32)
    gtable[V:, 2] = np.asarray(inputs["var_has_outer"], f32)
    gtable[V:, 3] = np.asarray(inputs["var_has_inner"], f32)
    gtable[V:, 4] = fam_id
    gtable[V:, 5] = np.asarray(inputs["var_group_id"], f32)

    W1 = np.asarray(inputs["W1"], f32)
    b1 = np.asarray(inputs["b1"], f32)
    W2 = np.asarray(inputs["W2"], f32)
    b2 = np.asarray(inputs["b2"], f32)
    gamma = np.asarray(inputs["ln_gamma"], f32)
    beta = np.asarray(inputs["ln_beta"], f32)
    scale = np.float32(np.asarray(inputs["scale"]))

    W1g = gamma[:, None] * W1
    w1e = np.concatenate([W1g[REF_PERM], (beta @ W1 + b1)[None]],
                         axis=0).astype(ml_dtypes.bfloat16)
    w2e = np.concatenate([W2 * scale, (b2 * scale)[None]],
                         axis=0).astype(ml_dtypes.bfloat16)

    cst_f32, cst_b16 = make_consts(L)

    in_maps = []
    NIQ = 2 * L // NQ
    cols = NIQ // 16
    ar = np.arange(NIQ)
    for c in range(n_cores):
        gidx = np.zeros((P, R, cols), np.int16)
        for r in range(R):
            flat = np.concatenate([
                tok_ids[c * R + r],
                var_ids[c * R + r].astype(np.int64) + V,
            ]).astype(np.int16)
            for q in range(NQ):
                chunk = flat[q * NIQ:(q + 1) * NIQ]
                w16 = np.zeros((16, cols), np.int16)
                w16[ar % 16, ar // 16] = chunk
                gidx[32 * q:32 * q + 16, r] = w16
                gidx[32 * q + 16:32 * q + 32, r] = w16
        in_maps.append({
            "gidx": gidx,
            "gtable": gtable,
            "w1e": w1e,
            "w2e": w2e,
            "cst_f32": cst_f32,
            "cst_b16": cst_b16,
        })
    return in_maps


_CACHE = {}


def _get_program(R, L):
    key = (R, L)
    if key not in _CACHE:
        _CACHE[key] = build_program(R, L)
    return _CACHE[key]


def kernel(**inputs):
    from concourse.bass_utils import run_bass_kernel_spmd

    B, L = np.asarray(inputs["token_ids"]).shape
    n_cores = 8
    R = B // n_cores
    nc = _get_program(R, L)
    in_maps = prep_host(inputs, n_cores, R, L)
    trace = bool(int(os.environ.get("KERNEL_TRACE", "0")))
    try:
        res = run_bass_kernel_spmd(nc, in_maps,
                                   core_ids=list(range(n_cores)),
                                   trace=trace)
    except Exception:
        if not trace:
            raise
        res = run_bass_kernel_spmd(nc, in_maps,
                                   core_ids=list(range(n_cores)),
                                   trace=False)
    kernel.last_results = res
    out = np.concatenate([np.asarray(r["out"]) for r in res.results], axis=0)
    return out.astype(np.float32)
